# revision 40
# baseline (speedup 1.0000x reference)
"""Trainium2 Bass kernel for nn_AverageAttention (B=8, L=2048, D=1024).

Math (per batch b):
    avg[t]  = cumsum(x, axis=t)[t] / (t+1)
    g       = concat([x, avg], -1) @ W_gate.T + b_gate        # (L, 2*D)
    out     = sigmoid(g[:, :D]) * x + sigmoid(g[:, D:]) * avg

Strategy: batch-parallel over 8 NeuronCores (one sequence per core), W_gate
replicated. On-chip layout is transposed (feature-on-partition,
token-on-free) so the cumulative sum is one DVE tensor_tensor_scan per
128-feature chunk.

The gating matmul runs in fp8-e4m3 with MatmulPerfMode.DoubleRow (two
128-row contraction chunks per instruction; measured ~795ns per
K=2048/N=512/M=128 accumulation group on HW vs ~3950ns for bf16). Contraction
chunk m pairs (x_m, avg_m). The whole W (4MB fp8) lives in SBUF, loaded once
per rep. Accuracy: fp8 operand quantization gives ~1.3e-2 rel on the gating
output (threshold 2e-2); avg path stays fp32-scan/bf16-store (~3e-4).

Outputs cross HBM as bf16 (halves store traffic; ~0.1% rounding), upcast to
fp32 on the host. All DMA rides the otherwise-idle sync (SP) HWDGE ring,
ordered: [W pair0 | invd | bias | x0..x7 | W rest (j-major) | avg stores |
gat stores] so x loads are never head-blocked.

Engine placement (HW-measured, not what the CoreSim cost model suggests):
every matmul group needs all 16 contraction chunks, so the kernel is gated
by when the last avg chunk's fp8 cast lands. The DVE (fastest engine) runs
only the scans pre-that-point plus the sigma_f*avg mul and final add of the
gate combine afterwards; Pool (gpsimd, slow per-op but absorbs heavy nominal
load) takes all fp8/bf16 casts, the cumsum*invd mul, and sigma_i*x; Act does
sigmoid evacuation ONLY - it is a single serial engine and any cast placed
ahead of the sigmoids in its in-order queue delays every PSUM evacuation
(that mistake cost +23us). Combine/store emission comes after the whole
phase-1 chain so the in-order queues never head-block the critical path.
"""

from contextlib import ExitStack

import ml_dtypes
import numpy as np

import concourse.bass as bass
import concourse.bass_utils as bass_utils
import concourse.mybir as mybir
import concourse.tile as tile
from concourse import bacc
from concourse._compat import with_exitstack
from concourse.bass import ts

B, L, D = 8, 2048, 1024
NJ = D // 128         # 8 feature chunks of x / avg
NOB = 2 * D // 128    # 16 output-feature blocks of g
NP = NJ               # 8 DoubleRow contraction pairs (x_m, avg_m)
TCW = 512             # matmul moving free-dim (1 PSUM bank)
NTC = L // TCW

FP32 = mybir.dt.float32
BF16 = mybir.dt.bfloat16
FP8 = mybir.dt.float8e4

F8NP = ml_dtypes.float8_e4m3
BFNP = ml_dtypes.bfloat16


@with_exitstack
def _tile_body(
    ctx: ExitStack,
    tc: tile.TileContext,
    reps: int = 1,
    no_mm: bool = False,
    no_act: bool = False,
    no_p1: bool = False,
    add_eng: str = "dve",
    minv_eng: str = "pool",
    scan_mode: str = "dve",
    fast_tail: bool = True,
    scan_bf: bool = False,
):
    nc = tc.nc

    xT = nc.dram_tensor("xT", (NJ, 128, L), FP32, kind="ExternalInput").ap()
    wq = nc.dram_tensor("wq", (128, NOB, NP, 2, 128), FP8, kind="ExternalInput").ap()
    invd = nc.dram_tensor("invd", (128, L), BF16, kind="ExternalInput").ap()
    biash = nc.dram_tensor("biash", (128, NOB), FP32, kind="ExternalInput").ap()
    avgT = nc.dram_tensor("avgT", (NJ, 128, L), BF16, kind="ExternalOutput").ap()
    gatT = nc.dram_tensor("gatT", (NJ, 128, L), BF16, kind="ExternalOutput").ap()

    const_pool = ctx.enter_context(tc.tile_pool(name="const", bufs=1))
    w_pool = ctx.enter_context(tc.tile_pool(name="w", bufs=1))
    cat_pool = ctx.enter_context(tc.tile_pool(name="cat", bufs=NP))
    abf_pool = ctx.enter_context(tc.tile_pool(name="abf", bufs=NJ))
    x_pool = ctx.enter_context(tc.tile_pool(name="x", bufs=NJ))
    ct_pool = ctx.enter_context(tc.tile_pool(name="ct", bufs=2))
    st_pool = ctx.enter_context(tc.tile_pool(name="st", bufs=3))
    gt_pool = ctx.enter_context(tc.tile_pool(name="gt", bufs=2))
    tmp_pool = ctx.enter_context(tc.tile_pool(name="tmp", bufs=1))
    psum_pool = ctx.enter_context(tc.tile_pool(name="psum", bufs=8, space="PSUM"))

    invd_sb = const_pool.tile([128, L], BF16, tag="invd")
    bias_sb = const_pool.tile([128, NOB], FP32, tag="bias")

    for _rep in range(reps):
        w_sb = w_pool.tile([128, NOB, NP, 2, 128], FP8, name="w_sb", tag="w_sb")
        cats = [
            cat_pool.tile([128, 2, L], FP8, tag="cat", name=f"cat{m}")
            for m in range(NP)
        ]
        abfs = [
            abf_pool.tile([128, L], BF16, tag="abf", name=f"abf{j}") for j in range(NJ)
        ]

        # --- sync-ring head: x0 first (the scan chain is the critical path),
        # then first W pair + constants, then the rest of x ---
        xts = []
        for j in range(NJ):
            xt = x_pool.tile([128, L], FP32, name="xt", tag="xt")
            nc.sync.dma_start(xt[:], xT[j])
            xts.append(xt)
            # Pool: x-half fp8 casts, paced only by the x DMAs
            nc.gpsimd.tensor_copy(cats[j][:, 0, :], xt[:])
            if j == 0:
                nc.sync.dma_start(w_sb[:, 0], wq[:, 0])
                nc.sync.dma_start(w_sb[:, NJ], wq[:, NJ])
                if _rep == 0:
                    nc.sync.dma_start(invd_sb[:], invd[:])
                    nc.sync.dma_start(bias_sb[:], biash[:])
        # remaining W, j-major so pair j's tiles land just before needed
        for j in range(1, NJ):
            nc.sync.dma_start(w_sb[:, j], wq[:, j])
            nc.sync.dma_start(w_sb[:, NJ + j], wq[:, NJ + j])

        # --- phase 1: the a8_7 critical chain.
        # DVE runs only scans; Pool only the avg muls; Act casts avg->fp8.
        # Everything else (combine, stores) is emitted after, so the in-order
        # queues never delay the last cat chunk the matmuls wait on.
        for j in range(NJ):
            xt = xts[j]
            if no_p1:
                nc.gpsimd.memset(cats[j][:], 0.25)
                nc.vector.tensor_copy(abfs[j][:], xt[:])
                nc.sync.dma_start(avgT[j], abfs[j][:])
                continue
            ct = ct_pool.tile([128, L], BF16 if scan_bf else FP32, name="ct", tag="ct")
            nc.vector.tensor_tensor_scan(
                ct[:], xt[:], xt[:], 0.0, mybir.AluOpType.add, mybir.AluOpType.bypass
            )
            if fast_tail and j == NJ - 1:
                # last chunk gates all matmul groups: produce its fp8 slot
                # directly on the DVE (one mul, fp8 out) instead of the
                # Pool mul->cast chain; the bf16 copy for store/combine is
                # off the critical path and follows on the DVE.
                nc.vector.tensor_mul(cats[j][:, 1, :], ct[:], invd_sb[:])
                nc.vector.tensor_mul(abfs[j][:], ct[:], invd_sb[:])
            elif minv_eng == "dve":
                nc.vector.tensor_mul(abfs[j][:], ct[:], invd_sb[:])
                nc.gpsimd.tensor_copy(cats[j][:, 1, :], abfs[j][:])
            else:
                nc.gpsimd.tensor_mul(abfs[j][:], ct[:], invd_sb[:])
                nc.gpsimd.tensor_copy(cats[j][:, 1, :], abfs[j][:])
            nc.sync.dma_start(avgT[j], abfs[j][:])

        if no_mm:
            for j in range(NJ):
                gt = gt_pool.tile([128, L], BF16, name="gt", tag="gt")
                nc.vector.tensor_mul(gt[:], xts[j][:], abfs[j][:])
                nc.sync.dma_start(gatT[j], gt[:])
            continue

        # --- phase 2: DoubleRow fp8 matmul, sigmoid evac, gate combine ---
        for j in range(NJ):
            sts = []
            for ob in (j, NJ + j):
                st = st_pool.tile([128, L], BF16, name="st", tag="st")
                for tcx in range(NTC):
                    s = ts(tcx, TCW)
                    ps = psum_pool.tile([128, TCW], FP32, name="ps", tag="ps")
                    for m in range(NP):
                        nc.tensor.matmul(
                            ps[:],
                            w_sb[:, ob, m],
                            cats[m][:, :, s],
                            start=(m == 0),
                            stop=(m == NP - 1),
                            perf_mode=mybir.MatmulPerfMode.DoubleRow,
                        )
                    if no_act:
                        nc.scalar.copy(st[:, s], ps[:])
                    else:
                        nc.scalar.activation(
                            st[:, s],
                            ps[:],
                            mybir.ActivationFunctionType.Sigmoid,
                            bias=bias_sb[:, ob : ob + 1],
                        )
                sts.append(st)
            st_i, st_f = sts
            gt = gt_pool.tile([128, L], BF16, name="gt", tag="gt")
            tmp = tmp_pool.tile([128, L], BF16, name="tmp", tag="tmp")
            if j == NJ - 1:
                # per-chunk tail to shorten the serial path after the last matmul
                for tcx in range(NTC):
                    s = ts(tcx, TCW)
                    nc.gpsimd.tensor_mul(tmp[:, s], st_i[:, s], xts[j][:, s])
                    nc.vector.tensor_mul(gt[:, s], st_f[:, s], abfs[j][:, s])
                    nc.vector.tensor_add(gt[:, s], gt[:, s], tmp[:, s])
                    nc.sync.dma_start(gatT[j][:, s], gt[:, s])
            else:
                nc.gpsimd.tensor_mul(tmp[:], st_i[:], xts[j][:])
                nc.vector.tensor_mul(gt[:], st_f[:], abfs[j][:])
                if add_eng == "pool":
                    nc.gpsimd.tensor_add(gt[:], gt[:], tmp[:])
                else:
                    nc.vector.tensor_add(gt[:], gt[:], tmp[:])
                nc.sync.dma_start(gatT[j], gt[:])


_CACHE: dict = {}


def build_nc(reps: int | None = None):
    import os as _os

    if reps is None:
        reps = int(_os.environ.get("KREPS", "1"))
    no_mm = _os.environ.get("KNOMM", "0") == "1"
    no_act = _os.environ.get("KNOACT", "0") == "1"
    no_p1 = _os.environ.get("KNOP1", "0") == "1"
    add_eng = _os.environ.get("KADD", "dve")
    minv_eng = _os.environ.get("KMINV", "pool")
    scan_mode = _os.environ.get("KSCAN", "dve")
    fast_tail = _os.environ.get("KFT", "1") == "1"
    scan_bf = _os.environ.get("KSCBF", "0") == "1"
    key = ("nc", reps, no_mm, no_act, no_p1, add_eng, minv_eng, scan_mode,
           fast_tail, scan_bf)
    if key not in _CACHE:
        nc = bacc.Bacc(
            "TRN2",
            target_bir_lowering=False,
            debug=False,
            enable_asserts=True,
            num_devices=B,
        )
        with tile.TileContext(nc) as t:
            _tile_body(
                t, reps=reps, no_mm=no_mm, no_act=no_act, no_p1=no_p1,
                add_eng=add_eng, minv_eng=minv_eng, scan_mode=scan_mode,
                fast_tail=fast_tail, scan_bf=scan_bf,
            )
        nc.compile()
        _CACHE[key] = nc
    return _CACHE[key]


# contraction chunk order: pair m slot 0 = x chunk m, slot 1 = avg chunk m
KC_LIST = [c for m in range(NP) for c in (m, NJ + m)]


def prep_shared(W_gate: np.ndarray, b_gate: np.ndarray):
    # wq[p, ob, m, s, o] = W_gate[128*ob + o, 128*kc(m,s) + p]
    arr = np.ascontiguousarray(W_gate.astype(np.float32)).T.reshape(16, 128, NOB, 128)
    # arr[KC_LIST] is (ms, p, ob, o) -> want (p, ob, ms, o)
    wq = np.ascontiguousarray(arr[KC_LIST].transpose(1, 2, 0, 3)).reshape(
        128, NOB, NP, 2, 128
    ).astype(F8NP)
    invd = np.ascontiguousarray(
        np.broadcast_to(
            (1.0 / np.arange(1, L + 1, dtype=np.float32))[None, :], (128, L)
        )
    ).astype(BFNP)
    biash = np.ascontiguousarray(b_gate.astype(np.float32).reshape(NOB, 128).T)
    return wq, invd, biash


def kernel(inputs: np.ndarray, W_gate: np.ndarray, b_gate: np.ndarray, **run_kwargs):
    inputs = np.asarray(inputs, dtype=np.float32)
    W_gate = np.asarray(W_gate, dtype=np.float32)
    b_gate = np.asarray(b_gate, dtype=np.float32)
    assert inputs.shape == (B, L, D)

    wq, invd, biash = prep_shared(W_gate, b_gate)
    in_maps = []
    for c in range(B):
        xT_c = np.ascontiguousarray(inputs[c].T).reshape(NJ, 128, L)
        in_maps.append({"xT": xT_c, "wq": wq, "invd": invd, "biash": biash})

    nc = build_nc()
    res = bass_utils.run_bass_kernel_spmd(
        nc, in_maps, core_ids=list(range(B)), **run_kwargs
    )

    gating = np.empty((B, L, D), dtype=np.float32)
    average = np.empty((B, L, D), dtype=np.float32)
    for c in range(B):
        gating[c] = res.results[c]["gatT"].astype(np.float32).reshape(D, L).T
        average[c] = res.results[c]["avgT"].astype(np.float32).reshape(D, L).T
    if run_kwargs:
        _CACHE["last_results"] = res
    return gating, average


# revision 51
# speedup vs baseline: 1.0022x; 1.0022x over previous
"""Trainium2 Bass kernel for nn_AverageAttention (B=8, L=2048, D=1024).

Math (per batch b):
    avg[t]  = cumsum(x, axis=t)[t] / (t+1)
    g       = concat([x, avg], -1) @ W_gate.T + b_gate        # (L, 2*D)
    out     = sigmoid(g[:, :D]) * x + sigmoid(g[:, D:]) * avg

Strategy: batch-parallel over 8 NeuronCores (one sequence per core), W_gate
replicated. On-chip layout is transposed (feature-on-partition,
token-on-free) so the cumulative sum is one DVE tensor_tensor_scan per
128-feature chunk.

The gating matmul runs in fp8-e4m3 with MatmulPerfMode.DoubleRow (two
128-row contraction chunks per instruction; measured ~795ns per
K=2048/N=512/M=128 accumulation group on HW vs ~3950ns for bf16). Contraction
chunk m pairs (x_m, avg_m). The whole W (4MB fp8) lives in SBUF, loaded once
per rep. Accuracy: fp8 operand quantization gives ~1.3e-2 rel on the gating
output (threshold 2e-2); avg path stays fp32-scan/bf16-store (~3e-4).

Outputs cross HBM as bf16 (halves store traffic; ~0.1% rounding), upcast to
fp32 on the host. All DMA rides the otherwise-idle sync (SP) HWDGE ring,
ordered: [W pair0 | invd | bias | x0..x7 | W rest (j-major) | avg stores |
gat stores] so x loads are never head-blocked.

Engine placement (HW-measured, not what the CoreSim cost model suggests):
every matmul group needs all 16 contraction chunks, so the kernel is gated
by when the last avg chunk's fp8 cast lands. The DVE (fastest engine) runs
only the scans pre-that-point plus the sigma_f*avg mul and final add of the
gate combine afterwards; Pool (gpsimd, slow per-op but absorbs heavy nominal
load) takes all fp8/bf16 casts, the cumsum*invd mul, and sigma_i*x; Act does
sigmoid evacuation ONLY - it is a single serial engine and any cast placed
ahead of the sigmoids in its in-order queue delays every PSUM evacuation
(that mistake cost +23us). Combine/store emission comes after the whole
phase-1 chain so the in-order queues never head-block the critical path.
"""

from contextlib import ExitStack

import ml_dtypes
import numpy as np

import concourse.bass as bass
import concourse.bass_utils as bass_utils
import concourse.mybir as mybir
import concourse.tile as tile
from concourse import bacc
from concourse._compat import with_exitstack
from concourse.bass import ts

B, L, D = 8, 2048, 1024
NJ = D // 128         # 8 feature chunks of x / avg
NOB = 2 * D // 128    # 16 output-feature blocks of g
NP = NJ               # 8 DoubleRow contraction pairs (x_m, avg_m)
TCW = 512             # matmul moving free-dim (1 PSUM bank)
NTC = L // TCW

FP32 = mybir.dt.float32
BF16 = mybir.dt.bfloat16
FP8 = mybir.dt.float8e4

F8NP = ml_dtypes.float8_e4m3
BFNP = ml_dtypes.bfloat16


@with_exitstack
def _tile_body(
    ctx: ExitStack,
    tc: tile.TileContext,
    reps: int = 1,
    no_mm: bool = False,
    no_act: bool = False,
    no_p1: bool = False,
    add_eng: str = "dve",
    minv_eng: str = "pool",
    scan_mode: str = "dve",
    fast_tail: bool = True,
    scan_bf: bool = False,
    ft_n: int = 1,
    x8_act: int = 0,
    a8_act: bool = False,
):
    nc = tc.nc

    xT = nc.dram_tensor("xT", (NJ, 128, L), FP32, kind="ExternalInput").ap()
    wq = nc.dram_tensor("wq", (128, NOB, NP, 2, 128), FP8, kind="ExternalInput").ap()
    invd = nc.dram_tensor("invd", (128, L), BF16, kind="ExternalInput").ap()
    biash = nc.dram_tensor("biash", (128, NOB), FP32, kind="ExternalInput").ap()
    avgT = nc.dram_tensor("avgT", (NJ, 128, L), BF16, kind="ExternalOutput").ap()
    gatT = nc.dram_tensor("gatT", (NJ, 128, L), BF16, kind="ExternalOutput").ap()

    const_pool = ctx.enter_context(tc.tile_pool(name="const", bufs=1))
    w_pool = ctx.enter_context(tc.tile_pool(name="w", bufs=1))
    cat_pool = ctx.enter_context(tc.tile_pool(name="cat", bufs=NP))
    abf_pool = ctx.enter_context(tc.tile_pool(name="abf", bufs=NJ))
    x_pool = ctx.enter_context(tc.tile_pool(name="x", bufs=NJ))
    ct_pool = ctx.enter_context(tc.tile_pool(name="ct", bufs=2))
    st_pool = ctx.enter_context(tc.tile_pool(name="st", bufs=3))
    gt_pool = ctx.enter_context(tc.tile_pool(name="gt", bufs=2))
    tmp_pool = ctx.enter_context(tc.tile_pool(name="tmp", bufs=1))
    psum_pool = ctx.enter_context(tc.tile_pool(name="psum", bufs=8, space="PSUM"))

    invd_sb = const_pool.tile([128, L], BF16, tag="invd")
    bias_sb = const_pool.tile([128, NOB], FP32, tag="bias")

    for _rep in range(reps):
        w_sb = w_pool.tile([128, NOB, NP, 2, 128], FP8, name="w_sb", tag="w_sb")
        cats = [
            cat_pool.tile([128, 2, L], FP8, tag="cat", name=f"cat{m}")
            for m in range(NP)
        ]
        abfs = [
            abf_pool.tile([128, L], BF16, tag="abf", name=f"abf{j}") for j in range(NJ)
        ]

        # --- sync-ring head: x0 first (the scan chain is the critical path),
        # then first W pair + constants, then the rest of x ---
        xts = []
        for j in range(NJ):
            xt = x_pool.tile([128, L], FP32, name="xt", tag="xt")
            nc.sync.dma_start(xt[:], xT[j])
            xts.append(xt)
            # x-half fp8 casts, paced only by the x DMAs. Act is idle until
            # the first sigmoid (~55us in), so it can absorb some to relieve
            # Pool's in-order queue.
            if j < x8_act:
                nc.scalar.copy(cats[j][:, 0, :], xt[:])
            else:
                nc.gpsimd.tensor_copy(cats[j][:, 0, :], xt[:])
            if j == 0:
                nc.sync.dma_start(w_sb[:, 0], wq[:, 0])
                nc.sync.dma_start(w_sb[:, NJ], wq[:, NJ])
                if _rep == 0:
                    nc.sync.dma_start(invd_sb[:], invd[:])
                    nc.sync.dma_start(bias_sb[:], biash[:])
        # remaining W, j-major so pair j's tiles land just before needed
        for j in range(1, NJ):
            nc.sync.dma_start(w_sb[:, j], wq[:, j])
            nc.sync.dma_start(w_sb[:, NJ + j], wq[:, NJ + j])

        # --- phase 1: the a8_7 critical chain.
        # DVE runs only scans; Pool only the avg muls; Act casts avg->fp8.
        # Everything else (combine, stores) is emitted after, so the in-order
        # queues never delay the last cat chunk the matmuls wait on.
        for j in range(NJ):
            xt = xts[j]
            if no_p1:
                nc.gpsimd.memset(cats[j][:], 0.25)
                nc.vector.tensor_copy(abfs[j][:], xt[:])
                nc.sync.dma_start(avgT[j], abfs[j][:])
                continue
            ct = ct_pool.tile([128, L], BF16 if scan_bf else FP32, name="ct", tag="ct")
            nc.vector.tensor_tensor_scan(
                ct[:], xt[:], xt[:], 0.0, mybir.AluOpType.add, mybir.AluOpType.bypass
            )
            if fast_tail and j >= NJ - ft_n:
                # last chunk gates all matmul groups: produce its fp8 slot
                # directly on the DVE (one mul, fp8 out) instead of the
                # Pool mul->cast chain; the bf16 copy for store/combine is
                # off the critical path and follows on the DVE.
                nc.vector.tensor_mul(cats[j][:, 1, :], ct[:], invd_sb[:])
                nc.vector.tensor_mul(abfs[j][:], ct[:], invd_sb[:])
            elif minv_eng == "dve":
                nc.vector.tensor_mul(abfs[j][:], ct[:], invd_sb[:])
                nc.gpsimd.tensor_copy(cats[j][:, 1, :], abfs[j][:])
            else:
                nc.gpsimd.tensor_mul(abfs[j][:], ct[:], invd_sb[:])
                if a8_act:
                    nc.scalar.copy(cats[j][:, 1, :], abfs[j][:])
                else:
                    nc.gpsimd.tensor_copy(cats[j][:, 1, :], abfs[j][:])
            nc.sync.dma_start(avgT[j], abfs[j][:])

        if no_mm:
            for j in range(NJ):
                gt = gt_pool.tile([128, L], BF16, name="gt", tag="gt")
                nc.vector.tensor_mul(gt[:], xts[j][:], abfs[j][:])
                nc.sync.dma_start(gatT[j], gt[:])
            continue

        # --- phase 2: DoubleRow fp8 matmul, sigmoid evac, gate combine ---
        for j in range(NJ):
            sts = []
            for ob in (j, NJ + j):
                st = st_pool.tile([128, L], BF16, name="st", tag="st")
                for tcx in range(NTC):
                    s = ts(tcx, TCW)
                    ps = psum_pool.tile([128, TCW], FP32, name="ps", tag="ps")
                    for m in range(NP):
                        nc.tensor.matmul(
                            ps[:],
                            w_sb[:, ob, m],
                            cats[m][:, :, s],
                            start=(m == 0),
                            stop=(m == NP - 1),
                            perf_mode=mybir.MatmulPerfMode.DoubleRow,
                        )
                    if no_act:
                        nc.scalar.copy(st[:, s], ps[:])
                    else:
                        nc.scalar.activation(
                            st[:, s],
                            ps[:],
                            mybir.ActivationFunctionType.Sigmoid,
                            bias=bias_sb[:, ob : ob + 1],
                        )
                sts.append(st)
            st_i, st_f = sts
            gt = gt_pool.tile([128, L], BF16, name="gt", tag="gt")
            tmp = tmp_pool.tile([128, L], BF16, name="tmp", tag="tmp")
            if j == NJ - 1:
                # per-chunk tail to shorten the serial path after the last matmul
                for tcx in range(NTC):
                    s = ts(tcx, TCW)
                    nc.gpsimd.tensor_mul(tmp[:, s], st_i[:, s], xts[j][:, s])
                    nc.vector.tensor_mul(gt[:, s], st_f[:, s], abfs[j][:, s])
                    nc.vector.tensor_add(gt[:, s], gt[:, s], tmp[:, s])
                    nc.sync.dma_start(gatT[j][:, s], gt[:, s])
            else:
                nc.gpsimd.tensor_mul(tmp[:], st_i[:], xts[j][:])
                nc.vector.tensor_mul(gt[:], st_f[:], abfs[j][:])
                if add_eng == "pool":
                    nc.gpsimd.tensor_add(gt[:], gt[:], tmp[:])
                else:
                    nc.vector.tensor_add(gt[:], gt[:], tmp[:])
                nc.sync.dma_start(gatT[j], gt[:])


_CACHE: dict = {}


def build_nc(reps: int | None = None):
    import os as _os

    if reps is None:
        reps = int(_os.environ.get("KREPS", "1"))
    no_mm = _os.environ.get("KNOMM", "0") == "1"
    no_act = _os.environ.get("KNOACT", "0") == "1"
    no_p1 = _os.environ.get("KNOP1", "0") == "1"
    add_eng = _os.environ.get("KADD", "dve")
    minv_eng = _os.environ.get("KMINV", "pool")
    scan_mode = _os.environ.get("KSCAN", "dve")
    fast_tail = _os.environ.get("KFT", "1") == "1"
    scan_bf = _os.environ.get("KSCBF", "0") == "1"
    ft_n = int(_os.environ.get("KFTN", "1"))
    x8_act = int(_os.environ.get("KX8A", "8"))
    a8_act = _os.environ.get("KA8A", "1") == "1"
    key = ("nc", reps, no_mm, no_act, no_p1, add_eng, minv_eng, scan_mode,
           fast_tail, scan_bf, ft_n, x8_act, a8_act)
    if key not in _CACHE:
        nc = bacc.Bacc(
            "TRN2",
            target_bir_lowering=False,
            debug=False,
            enable_asserts=True,
            num_devices=B,
        )
        with tile.TileContext(nc) as t:
            _tile_body(
                t, reps=reps, no_mm=no_mm, no_act=no_act, no_p1=no_p1,
                add_eng=add_eng, minv_eng=minv_eng, scan_mode=scan_mode,
                fast_tail=fast_tail, scan_bf=scan_bf, ft_n=ft_n,
                x8_act=x8_act, a8_act=a8_act,
            )
        nc.compile()
        _CACHE[key] = nc
    return _CACHE[key]


# contraction chunk order: pair m slot 0 = x chunk m, slot 1 = avg chunk m
KC_LIST = [c for m in range(NP) for c in (m, NJ + m)]


def prep_shared(W_gate: np.ndarray, b_gate: np.ndarray):
    # wq[p, ob, m, s, o] = W_gate[128*ob + o, 128*kc(m,s) + p]
    arr = np.ascontiguousarray(W_gate.astype(np.float32)).T.reshape(16, 128, NOB, 128)
    # arr[KC_LIST] is (ms, p, ob, o) -> want (p, ob, ms, o)
    wq = np.ascontiguousarray(arr[KC_LIST].transpose(1, 2, 0, 3)).reshape(
        128, NOB, NP, 2, 128
    ).astype(F8NP)
    invd = np.ascontiguousarray(
        np.broadcast_to(
            (1.0 / np.arange(1, L + 1, dtype=np.float32))[None, :], (128, L)
        )
    ).astype(BFNP)
    biash = np.ascontiguousarray(b_gate.astype(np.float32).reshape(NOB, 128).T)
    return wq, invd, biash


def kernel(inputs: np.ndarray, W_gate: np.ndarray, b_gate: np.ndarray, **run_kwargs):
    inputs = np.asarray(inputs, dtype=np.float32)
    W_gate = np.asarray(W_gate, dtype=np.float32)
    b_gate = np.asarray(b_gate, dtype=np.float32)
    assert inputs.shape == (B, L, D)

    wq, invd, biash = prep_shared(W_gate, b_gate)
    in_maps = []
    for c in range(B):
        xT_c = np.ascontiguousarray(inputs[c].T).reshape(NJ, 128, L)
        in_maps.append({"xT": xT_c, "wq": wq, "invd": invd, "biash": biash})

    nc = build_nc()
    res = bass_utils.run_bass_kernel_spmd(
        nc, in_maps, core_ids=list(range(B)), **run_kwargs
    )

    gating = np.empty((B, L, D), dtype=np.float32)
    average = np.empty((B, L, D), dtype=np.float32)
    for c in range(B):
        gating[c] = res.results[c]["gatT"].astype(np.float32).reshape(D, L).T
        average[c] = res.results[c]["avgT"].astype(np.float32).reshape(D, L).T
    if run_kwargs:
        _CACHE["last_results"] = res
    return gating, average


# revision 57
# speedup vs baseline: 1.0101x; 1.0079x over previous
"""Trainium2 Bass kernel for nn_AverageAttention (B=8, L=2048, D=1024).

Math (per batch b):
    avg[t]  = cumsum(x, axis=t)[t] / (t+1)
    g       = concat([x, avg], -1) @ W_gate.T + b_gate        # (L, 2*D)
    out     = sigmoid(g[:, :D]) * x + sigmoid(g[:, D:]) * avg

Strategy: batch-parallel over 8 NeuronCores (one sequence per core), W_gate
replicated. On-chip layout is transposed (feature-on-partition,
token-on-free) so the cumulative sum is one DVE tensor_tensor_scan per
128-feature chunk.

The gating matmul runs in fp8-e4m3 with MatmulPerfMode.DoubleRow (two
128-row contraction chunks per instruction; measured ~795ns per
K=2048/N=512/M=128 accumulation group on HW vs ~3950ns for bf16). Contraction
chunk m pairs (x_m, avg_m). The whole W (4MB fp8) lives in SBUF, loaded once
per rep. Accuracy: fp8 operand quantization gives ~1.3e-2 rel on the gating
output (threshold 2e-2); avg path stays fp32-scan/bf16-store (~3e-4).

Outputs cross HBM as bf16 (halves store traffic; ~0.1% rounding), upcast to
fp32 on the host. All DMA rides the otherwise-idle sync (SP) HWDGE ring,
ordered: [W pair0 | invd | bias | x0..x7 | W rest (j-major) | avg stores |
gat stores] so x loads are never head-blocked.

Engine placement (HW-measured; queue POSITION matters more than nominal op
cost): every matmul group needs all 16 contraction chunks, so the kernel is
gated by when the last avg chunks' fp8 casts land. DVE runs the scans and,
for the final chunk, produces its fp8 slot directly (ct*invd mul, fp8 out)
so it never waits on Pool's queue; Pool runs only the cumsum*invd muls and
the sigma_i*x combine mul; Act runs both families of fp8 casts (x8 + a8) -
it is idle until the first sigmoid (~55us in), whereas on Pool those casts
sat AHEAD of every avg-chain op in its in-order queue and paced the whole
kernel. The sigma_f*avg mul and final add run on DVE after the scans.
Combine/store emission comes after the whole phase-1 chain so the in-order
queues never head-block the critical path.
"""

from contextlib import ExitStack

import ml_dtypes
import numpy as np

import concourse.bass as bass
import concourse.bass_utils as bass_utils
import concourse.mybir as mybir
import concourse.tile as tile
from concourse import bacc
from concourse._compat import with_exitstack
from concourse.bass import ts

B, L, D = 8, 2048, 1024
NJ = D // 128         # 8 feature chunks of x / avg
NOB = 2 * D // 128    # 16 output-feature blocks of g
NP = NJ               # 8 DoubleRow contraction pairs (x_m, avg_m)
TCW = 512             # matmul moving free-dim (1 PSUM bank)
NTC = L // TCW

FP32 = mybir.dt.float32
BF16 = mybir.dt.bfloat16
FP8 = mybir.dt.float8e4

F8NP = ml_dtypes.float8_e4m3
BFNP = ml_dtypes.bfloat16


@with_exitstack
def _tile_body(
    ctx: ExitStack,
    tc: tile.TileContext,
    reps: int = 1,
    no_mm: bool = False,
    no_act: bool = False,
    no_p1: bool = False,
    add_eng: str = "dve",
    minv_eng: str = "pool",
    scan_mode: str = "dve",
    fast_tail: bool = True,
    scan_bf: bool = False,
    ft_n: int = 1,
    x8_act: int = 0,
    a8_act: bool = False,
    st_bufs: int = 3,
):
    nc = tc.nc

    xT = nc.dram_tensor("xT", (NJ, 128, L), FP32, kind="ExternalInput").ap()
    wq = nc.dram_tensor("wq", (128, NOB, NP, 2, 128), FP8, kind="ExternalInput").ap()
    invd = nc.dram_tensor("invd", (128, L), BF16, kind="ExternalInput").ap()
    biash = nc.dram_tensor("biash", (128, NOB), FP32, kind="ExternalInput").ap()
    avgT = nc.dram_tensor("avgT", (NJ, 128, L), BF16, kind="ExternalOutput").ap()
    gatT = nc.dram_tensor("gatT", (NJ, 128, L), BF16, kind="ExternalOutput").ap()

    const_pool = ctx.enter_context(tc.tile_pool(name="const", bufs=1))
    w_pool = ctx.enter_context(tc.tile_pool(name="w", bufs=1))
    cat_pool = ctx.enter_context(tc.tile_pool(name="cat", bufs=NP))
    abf_pool = ctx.enter_context(tc.tile_pool(name="abf", bufs=NJ))
    x_pool = ctx.enter_context(tc.tile_pool(name="x", bufs=NJ))
    ct_pool = ctx.enter_context(tc.tile_pool(name="ct", bufs=2))
    st_pool = ctx.enter_context(tc.tile_pool(name="st", bufs=st_bufs))
    gt_pool = ctx.enter_context(
        tc.tile_pool(name="gt", bufs=1 if st_bufs >= 4 else 2)
    )
    tmp_pool = ctx.enter_context(tc.tile_pool(name="tmp", bufs=1))
    psum_pool = ctx.enter_context(tc.tile_pool(name="psum", bufs=8, space="PSUM"))

    invd_sb = const_pool.tile([128, L], BF16, tag="invd")
    bias_sb = const_pool.tile([128, NOB], FP32, tag="bias")

    for _rep in range(reps):
        w_sb = w_pool.tile([128, NOB, NP, 2, 128], FP8, name="w_sb", tag="w_sb")
        cats = [
            cat_pool.tile([128, 2, L], FP8, tag="cat", name=f"cat{m}")
            for m in range(NP)
        ]
        abfs = [
            abf_pool.tile([128, L], BF16, tag="abf", name=f"abf{j}") for j in range(NJ)
        ]

        # --- sync-ring head: x0 first (the scan chain is the critical path),
        # then first W pair + constants, then the rest of x ---
        xts = []
        for j in range(NJ):
            xt = x_pool.tile([128, L], FP32, name="xt", tag="xt")
            nc.sync.dma_start(xt[:], xT[j])
            xts.append(xt)
            # x-half fp8 casts, paced only by the x DMAs. Act is idle until
            # the first sigmoid (~55us in), so it can absorb some to relieve
            # Pool's in-order queue.
            if j < x8_act:
                nc.scalar.copy(cats[j][:, 0, :], xt[:])
            else:
                nc.gpsimd.tensor_copy(cats[j][:, 0, :], xt[:])
            if j == 0:
                nc.sync.dma_start(w_sb[:, 0], wq[:, 0])
                nc.sync.dma_start(w_sb[:, NJ], wq[:, NJ])
                if _rep == 0:
                    nc.sync.dma_start(invd_sb[:], invd[:])
                    nc.sync.dma_start(bias_sb[:], biash[:])
        # remaining W, j-major so pair j's tiles land just before needed
        for j in range(1, NJ):
            nc.sync.dma_start(w_sb[:, j], wq[:, j])
            nc.sync.dma_start(w_sb[:, NJ + j], wq[:, NJ + j])

        # --- phase 1: the a8_7 critical chain.
        # DVE runs only scans; Pool only the avg muls; Act casts avg->fp8.
        # Everything else (combine, stores) is emitted after, so the in-order
        # queues never delay the last cat chunk the matmuls wait on.
        for j in range(NJ):
            xt = xts[j]
            if no_p1:
                nc.gpsimd.memset(cats[j][:], 0.25)
                nc.vector.tensor_copy(abfs[j][:], xt[:])
                nc.sync.dma_start(avgT[j], abfs[j][:])
                continue
            ct = ct_pool.tile([128, L], BF16 if scan_bf else FP32, name="ct", tag="ct")
            nc.vector.tensor_tensor_scan(
                ct[:], xt[:], xt[:], 0.0, mybir.AluOpType.add, mybir.AluOpType.bypass
            )
            if fast_tail and j >= NJ - ft_n:
                # last chunk gates all matmul groups: produce its fp8 slot
                # directly on the DVE (one mul, fp8 out) instead of the
                # Pool mul->cast chain; the bf16 copy for store/combine is
                # off the critical path and follows on the DVE.
                nc.vector.tensor_mul(cats[j][:, 1, :], ct[:], invd_sb[:])
                nc.vector.tensor_mul(abfs[j][:], ct[:], invd_sb[:])
            elif minv_eng == "dve":
                nc.vector.tensor_mul(abfs[j][:], ct[:], invd_sb[:])
                nc.gpsimd.tensor_copy(cats[j][:, 1, :], abfs[j][:])
            else:
                nc.gpsimd.tensor_mul(abfs[j][:], ct[:], invd_sb[:])
                if a8_act:
                    nc.scalar.copy(cats[j][:, 1, :], abfs[j][:])
                else:
                    nc.gpsimd.tensor_copy(cats[j][:, 1, :], abfs[j][:])
            nc.sync.dma_start(avgT[j], abfs[j][:])

        if no_mm:
            for j in range(NJ):
                gt = gt_pool.tile([128, L], BF16, name="gt", tag="gt")
                nc.vector.tensor_mul(gt[:], xts[j][:], abfs[j][:])
                nc.sync.dma_start(gatT[j], gt[:])
            continue

        # --- phase 2: DoubleRow fp8 matmul, sigmoid evac, gate combine ---
        for j in range(NJ):
            sts = []
            for ob in (j, NJ + j):
                st = st_pool.tile([128, L], BF16, name="st", tag="st")
                for tcx in range(NTC):
                    s = ts(tcx, TCW)
                    ps = psum_pool.tile([128, TCW], FP32, name="ps", tag="ps")
                    for m in range(NP):
                        nc.tensor.matmul(
                            ps[:],
                            w_sb[:, ob, m],
                            cats[m][:, :, s],
                            start=(m == 0),
                            stop=(m == NP - 1),
                            perf_mode=mybir.MatmulPerfMode.DoubleRow,
                        )
                    if no_act:
                        nc.scalar.copy(st[:, s], ps[:])
                    else:
                        nc.scalar.activation(
                            st[:, s],
                            ps[:],
                            mybir.ActivationFunctionType.Sigmoid,
                            bias=bias_sb[:, ob : ob + 1],
                        )
                sts.append(st)
            st_i, st_f = sts
            gt = gt_pool.tile([128, L], BF16, name="gt", tag="gt")
            tmp = tmp_pool.tile([128, L], BF16, name="tmp", tag="tmp")
            if j == NJ - 1:
                # per-chunk tail to shorten the serial path after the last matmul
                for tcx in range(NTC):
                    s = ts(tcx, TCW)
                    nc.gpsimd.tensor_mul(tmp[:, s], st_i[:, s], xts[j][:, s])
                    nc.vector.tensor_mul(gt[:, s], st_f[:, s], abfs[j][:, s])
                    nc.vector.tensor_add(gt[:, s], gt[:, s], tmp[:, s])
                    nc.sync.dma_start(gatT[j][:, s], gt[:, s])
            else:
                nc.gpsimd.tensor_mul(tmp[:], st_i[:], xts[j][:])
                nc.vector.tensor_mul(gt[:], st_f[:], abfs[j][:])
                if add_eng == "pool":
                    nc.gpsimd.tensor_add(gt[:], gt[:], tmp[:])
                else:
                    nc.vector.tensor_add(gt[:], gt[:], tmp[:])
                nc.sync.dma_start(gatT[j], gt[:])


_CACHE: dict = {}


def build_nc(reps: int | None = None):
    import os as _os

    if reps is None:
        reps = int(_os.environ.get("KREPS", "1"))
    no_mm = _os.environ.get("KNOMM", "0") == "1"
    no_act = _os.environ.get("KNOACT", "0") == "1"
    no_p1 = _os.environ.get("KNOP1", "0") == "1"
    add_eng = _os.environ.get("KADD", "dve")
    minv_eng = _os.environ.get("KMINV", "pool")
    scan_mode = _os.environ.get("KSCAN", "dve")
    fast_tail = _os.environ.get("KFT", "1") == "1"
    scan_bf = _os.environ.get("KSCBF", "0") == "1"
    ft_n = int(_os.environ.get("KFTN", "1"))
    x8_act = int(_os.environ.get("KX8A", "8"))
    a8_act = _os.environ.get("KA8A", "1") == "1"
    st_bufs = int(_os.environ.get("KSTB", "3"))
    key = ("nc", reps, no_mm, no_act, no_p1, add_eng, minv_eng, scan_mode,
           fast_tail, scan_bf, ft_n, x8_act, a8_act, st_bufs)
    if key not in _CACHE:
        nc = bacc.Bacc(
            "TRN2",
            target_bir_lowering=False,
            debug=False,
            enable_asserts=True,
            num_devices=B,
        )
        with tile.TileContext(nc) as t:
            _tile_body(
                t, reps=reps, no_mm=no_mm, no_act=no_act, no_p1=no_p1,
                add_eng=add_eng, minv_eng=minv_eng, scan_mode=scan_mode,
                fast_tail=fast_tail, scan_bf=scan_bf, ft_n=ft_n,
                x8_act=x8_act, a8_act=a8_act, st_bufs=st_bufs,
            )
        nc.compile()
        _CACHE[key] = nc
    return _CACHE[key]


# contraction chunk order: pair m slot 0 = x chunk m, slot 1 = avg chunk m
KC_LIST = [c for m in range(NP) for c in (m, NJ + m)]


def prep_shared(W_gate: np.ndarray, b_gate: np.ndarray):
    # wq[p, ob, m, s, o] = W_gate[128*ob + o, 128*kc(m,s) + p]
    arr = np.ascontiguousarray(W_gate.astype(np.float32)).T.reshape(16, 128, NOB, 128)
    # arr[KC_LIST] is (ms, p, ob, o) -> want (p, ob, ms, o)
    wq = np.ascontiguousarray(arr[KC_LIST].transpose(1, 2, 0, 3)).reshape(
        128, NOB, NP, 2, 128
    ).astype(F8NP)
    invd = np.ascontiguousarray(
        np.broadcast_to(
            (1.0 / np.arange(1, L + 1, dtype=np.float32))[None, :], (128, L)
        )
    ).astype(BFNP)
    biash = np.ascontiguousarray(b_gate.astype(np.float32).reshape(NOB, 128).T)
    return wq, invd, biash


def kernel(inputs: np.ndarray, W_gate: np.ndarray, b_gate: np.ndarray, **run_kwargs):
    inputs = np.asarray(inputs, dtype=np.float32)
    W_gate = np.asarray(W_gate, dtype=np.float32)
    b_gate = np.asarray(b_gate, dtype=np.float32)
    assert inputs.shape == (B, L, D)

    wq, invd, biash = prep_shared(W_gate, b_gate)
    in_maps = []
    for c in range(B):
        xT_c = np.ascontiguousarray(inputs[c].T).reshape(NJ, 128, L)
        in_maps.append({"xT": xT_c, "wq": wq, "invd": invd, "biash": biash})

    nc = build_nc()
    res = bass_utils.run_bass_kernel_spmd(
        nc, in_maps, core_ids=list(range(B)), **run_kwargs
    )

    gating = np.empty((B, L, D), dtype=np.float32)
    average = np.empty((B, L, D), dtype=np.float32)
    for c in range(B):
        gating[c] = res.results[c]["gatT"].astype(np.float32).reshape(D, L).T
        average[c] = res.results[c]["avgT"].astype(np.float32).reshape(D, L).T
    if run_kwargs:
        _CACHE["last_results"] = res
    return gating, average


# revision 58
# speedup vs baseline: 1.0344x; 1.0241x over previous
"""Trainium2 Bass kernel for nn_AverageAttention (B=8, L=2048, D=1024).

Math (per batch b):
    avg[t]  = cumsum(x, axis=t)[t] / (t+1)
    g       = concat([x, avg], -1) @ W_gate.T + b_gate        # (L, 2*D)
    out     = sigmoid(g[:, :D]) * x + sigmoid(g[:, D:]) * avg

Strategy: batch-parallel over 8 NeuronCores (one sequence per core), W_gate
replicated. On-chip layout is transposed (feature-on-partition,
token-on-free) so the cumulative sum is one DVE tensor_tensor_scan per
128-feature chunk.

The gating matmul runs in fp8-e4m3 with MatmulPerfMode.DoubleRow (two
128-row contraction chunks per instruction; measured ~795ns per
K=2048/N=512/M=128 accumulation group on HW vs ~3950ns for bf16). Contraction
chunk m pairs (x_m, avg_m). The whole W (4MB fp8) lives in SBUF, loaded once
per rep. Accuracy: fp8 operand quantization gives ~1.3e-2 rel on the gating
output (threshold 2e-2); avg path stays fp32-scan/bf16-store (~3e-4).

Outputs cross HBM as bf16 (halves store traffic; ~0.1% rounding), upcast to
fp32 on the host. All DMA rides the otherwise-idle sync (SP) HWDGE ring,
ordered: [W pair0 | invd | bias | x0..x7 | W rest (j-major) | avg stores |
gat stores] so x loads are never head-blocked.

Engine placement (HW-measured; queue POSITION matters more than nominal op
cost): every matmul group needs all 16 contraction chunks, so the kernel is
gated by when the last avg chunks' fp8 casts land. DVE runs the scans and,
for the final chunk, produces its fp8 slot directly (ct*invd mul, fp8 out)
so it never waits on Pool's queue; Pool runs only the cumsum*invd muls and
the sigma_i*x combine mul; Act runs both families of fp8 casts (x8 + a8) -
it is idle until the first sigmoid (~55us in), whereas on Pool those casts
sat AHEAD of every avg-chain op in its in-order queue and paced the whole
kernel. The sigma_f*avg mul and final add run on DVE after the scans.
Combine/store emission comes after the whole phase-1 chain so the in-order
queues never head-block the critical path.
"""

from contextlib import ExitStack

import ml_dtypes
import numpy as np

import concourse.bass as bass
import concourse.bass_utils as bass_utils
import concourse.mybir as mybir
import concourse.tile as tile
from concourse import bacc
from concourse._compat import with_exitstack
from concourse.bass import ts

B, L, D = 8, 2048, 1024
NJ = D // 128         # 8 feature chunks of x / avg
NOB = 2 * D // 128    # 16 output-feature blocks of g
NP = NJ               # 8 DoubleRow contraction pairs (x_m, avg_m)
TCW = 512             # matmul moving free-dim (1 PSUM bank)
NTC = L // TCW

FP32 = mybir.dt.float32
BF16 = mybir.dt.bfloat16
FP8 = mybir.dt.float8e4

F8NP = ml_dtypes.float8_e4m3
BFNP = ml_dtypes.bfloat16


@with_exitstack
def _tile_body(
    ctx: ExitStack,
    tc: tile.TileContext,
    reps: int = 1,
    no_mm: bool = False,
    no_act: bool = False,
    no_p1: bool = False,
    add_eng: str = "dve",
    minv_eng: str = "pool",
    scan_mode: str = "dve",
    fast_tail: bool = True,
    scan_bf: bool = False,
    ft_n: int = 1,
    x8_act: int = 0,
    a8_act: bool = False,
    st_bufs: int = 3,
):
    nc = tc.nc

    xT = nc.dram_tensor("xT", (NJ, 128, L), FP32, kind="ExternalInput").ap()
    wq = nc.dram_tensor("wq", (128, NOB, NP, 2, 128), FP8, kind="ExternalInput").ap()
    invd = nc.dram_tensor("invd", (128, L), BF16, kind="ExternalInput").ap()
    biash = nc.dram_tensor("biash", (128, NOB), FP32, kind="ExternalInput").ap()
    avgT = nc.dram_tensor("avgT", (NJ, 128, L), BF16, kind="ExternalOutput").ap()
    gatT = nc.dram_tensor("gatT", (NJ, 128, L), BF16, kind="ExternalOutput").ap()

    const_pool = ctx.enter_context(tc.tile_pool(name="const", bufs=1))
    w_pool = ctx.enter_context(tc.tile_pool(name="w", bufs=1))
    cat_pool = ctx.enter_context(tc.tile_pool(name="cat", bufs=NP))
    abf_pool = ctx.enter_context(tc.tile_pool(name="abf", bufs=NJ))
    x_pool = ctx.enter_context(tc.tile_pool(name="x", bufs=NJ))
    ct_pool = ctx.enter_context(tc.tile_pool(name="ct", bufs=2))
    st_pool = ctx.enter_context(tc.tile_pool(name="st", bufs=st_bufs))
    gt_pool = ctx.enter_context(
        tc.tile_pool(name="gt", bufs=1 if st_bufs >= 4 else 2)
    )
    tmp_pool = ctx.enter_context(tc.tile_pool(name="tmp", bufs=1))
    psum_pool = ctx.enter_context(tc.tile_pool(name="psum", bufs=8, space="PSUM"))

    invd_sb = const_pool.tile([128, L], BF16, tag="invd")
    bias_sb = const_pool.tile([128, NOB], FP32, tag="bias")

    for _rep in range(reps):
        w_sb = w_pool.tile([128, NOB, NP, 2, 128], FP8, name="w_sb", tag="w_sb")
        cats = [
            cat_pool.tile([128, 2, L], FP8, tag="cat", name=f"cat{m}")
            for m in range(NP)
        ]
        abfs = [
            abf_pool.tile([128, L], BF16, tag="abf", name=f"abf{j}") for j in range(NJ)
        ]

        # --- sync-ring head: x0 first (the scan chain is the critical path),
        # then first W pair + constants, then the rest of x ---
        xts = []
        for j in range(NJ):
            xt = x_pool.tile([128, L], FP32, name="xt", tag="xt")
            nc.sync.dma_start(xt[:], xT[j])
            xts.append(xt)
            # x-half fp8 casts, paced only by the x DMAs. Act is idle until
            # the first sigmoid (~55us in), so it can absorb some to relieve
            # Pool's in-order queue.
            if j < x8_act:
                nc.scalar.copy(cats[j][:, 0, :], xt[:])
            else:
                nc.gpsimd.tensor_copy(cats[j][:, 0, :], xt[:])
            if j == 0:
                nc.sync.dma_start(w_sb[:, 0], wq[:, 0])
                nc.sync.dma_start(w_sb[:, NJ], wq[:, NJ])
                if _rep == 0:
                    nc.sync.dma_start(invd_sb[:], invd[:])
                    nc.sync.dma_start(bias_sb[:], biash[:])
        # remaining W, j-major so pair j's tiles land just before needed
        for j in range(1, NJ):
            nc.sync.dma_start(w_sb[:, j], wq[:, j])
            nc.sync.dma_start(w_sb[:, NJ + j], wq[:, NJ + j])

        # --- phase 1: the a8_7 critical chain.
        # DVE runs only scans; Pool only the avg muls; Act casts avg->fp8.
        # Everything else (combine, stores) is emitted after, so the in-order
        # queues never delay the last cat chunk the matmuls wait on.
        for j in range(NJ):
            xt = xts[j]
            if no_p1:
                nc.gpsimd.memset(cats[j][:], 0.25)
                nc.vector.tensor_copy(abfs[j][:], xt[:])
                nc.sync.dma_start(avgT[j], abfs[j][:])
                continue
            ct = ct_pool.tile([128, L], BF16 if scan_bf else FP32, name="ct", tag="ct")
            nc.vector.tensor_tensor_scan(
                ct[:], xt[:], xt[:], 0.0, mybir.AluOpType.add, mybir.AluOpType.bypass
            )
            if fast_tail and j >= NJ - ft_n:
                # last chunk gates all matmul groups: produce its fp8 slot
                # directly on the DVE (one mul, fp8 out) instead of the
                # Pool mul->cast chain; the bf16 copy for store/combine is
                # off the critical path and follows on the DVE.
                nc.vector.tensor_mul(cats[j][:, 1, :], ct[:], invd_sb[:])
                nc.vector.tensor_mul(abfs[j][:], ct[:], invd_sb[:])
            elif minv_eng == "dve":
                nc.vector.tensor_mul(abfs[j][:], ct[:], invd_sb[:])
                nc.gpsimd.tensor_copy(cats[j][:, 1, :], abfs[j][:])
            else:
                nc.gpsimd.tensor_mul(abfs[j][:], ct[:], invd_sb[:])
                if a8_act:
                    nc.scalar.copy(cats[j][:, 1, :], abfs[j][:])
                else:
                    nc.gpsimd.tensor_copy(cats[j][:, 1, :], abfs[j][:])
            nc.sync.dma_start(avgT[j], abfs[j][:])

        if no_mm:
            for j in range(NJ):
                gt = gt_pool.tile([128, L], BF16, name="gt", tag="gt")
                nc.vector.tensor_mul(gt[:], xts[j][:], abfs[j][:])
                nc.sync.dma_start(gatT[j], gt[:])
            continue

        # --- phase 2: DoubleRow fp8 matmul, sigmoid evac, gate combine ---
        for j in range(NJ):
            sts = []
            for ob in (j, NJ + j):
                st = st_pool.tile([128, L], BF16, name="st", tag="st")
                for tcx in range(NTC):
                    s = ts(tcx, TCW)
                    ps = psum_pool.tile([128, TCW], FP32, name="ps", tag="ps")
                    for m in range(NP):
                        nc.tensor.matmul(
                            ps[:],
                            w_sb[:, ob, m],
                            cats[m][:, :, s],
                            start=(m == 0),
                            stop=(m == NP - 1),
                            perf_mode=mybir.MatmulPerfMode.DoubleRow,
                        )
                    if no_act:
                        nc.scalar.copy(st[:, s], ps[:])
                    else:
                        nc.scalar.activation(
                            st[:, s],
                            ps[:],
                            mybir.ActivationFunctionType.Sigmoid,
                            bias=bias_sb[:, ob : ob + 1],
                        )
                sts.append(st)
            st_i, st_f = sts
            gt = gt_pool.tile([128, L], BF16, name="gt", tag="gt")
            tmp = tmp_pool.tile([128, L], BF16, name="tmp", tag="tmp")
            if j == NJ - 1:
                # per-chunk tail to shorten the serial path after the last matmul
                for tcx in range(NTC):
                    s = ts(tcx, TCW)
                    nc.gpsimd.tensor_mul(tmp[:, s], st_i[:, s], xts[j][:, s])
                    nc.vector.tensor_mul(gt[:, s], st_f[:, s], abfs[j][:, s])
                    nc.vector.tensor_add(gt[:, s], gt[:, s], tmp[:, s])
                    nc.sync.dma_start(gatT[j][:, s], gt[:, s])
            else:
                nc.gpsimd.tensor_mul(tmp[:], st_i[:], xts[j][:])
                nc.vector.tensor_mul(gt[:], st_f[:], abfs[j][:])
                if add_eng == "pool":
                    nc.gpsimd.tensor_add(gt[:], gt[:], tmp[:])
                else:
                    nc.vector.tensor_add(gt[:], gt[:], tmp[:])
                nc.sync.dma_start(gatT[j], gt[:])


_CACHE: dict = {}


def build_nc(reps: int | None = None):
    import os as _os

    if reps is None:
        reps = int(_os.environ.get("KREPS", "1"))
    no_mm = _os.environ.get("KNOMM", "0") == "1"
    no_act = _os.environ.get("KNOACT", "0") == "1"
    no_p1 = _os.environ.get("KNOP1", "0") == "1"
    add_eng = _os.environ.get("KADD", "pool")
    minv_eng = _os.environ.get("KMINV", "pool")
    scan_mode = _os.environ.get("KSCAN", "dve")
    fast_tail = _os.environ.get("KFT", "1") == "1"
    scan_bf = _os.environ.get("KSCBF", "0") == "1"
    ft_n = int(_os.environ.get("KFTN", "1"))
    x8_act = int(_os.environ.get("KX8A", "8"))
    a8_act = _os.environ.get("KA8A", "1") == "1"
    st_bufs = int(_os.environ.get("KSTB", "3"))
    key = ("nc", reps, no_mm, no_act, no_p1, add_eng, minv_eng, scan_mode,
           fast_tail, scan_bf, ft_n, x8_act, a8_act, st_bufs)
    if key not in _CACHE:
        nc = bacc.Bacc(
            "TRN2",
            target_bir_lowering=False,
            debug=False,
            enable_asserts=True,
            num_devices=B,
        )
        with tile.TileContext(nc) as t:
            _tile_body(
                t, reps=reps, no_mm=no_mm, no_act=no_act, no_p1=no_p1,
                add_eng=add_eng, minv_eng=minv_eng, scan_mode=scan_mode,
                fast_tail=fast_tail, scan_bf=scan_bf, ft_n=ft_n,
                x8_act=x8_act, a8_act=a8_act, st_bufs=st_bufs,
            )
        nc.compile()
        _CACHE[key] = nc
    return _CACHE[key]


# contraction chunk order: pair m slot 0 = x chunk m, slot 1 = avg chunk m
KC_LIST = [c for m in range(NP) for c in (m, NJ + m)]


def prep_shared(W_gate: np.ndarray, b_gate: np.ndarray):
    # wq[p, ob, m, s, o] = W_gate[128*ob + o, 128*kc(m,s) + p]
    arr = np.ascontiguousarray(W_gate.astype(np.float32)).T.reshape(16, 128, NOB, 128)
    # arr[KC_LIST] is (ms, p, ob, o) -> want (p, ob, ms, o)
    wq = np.ascontiguousarray(arr[KC_LIST].transpose(1, 2, 0, 3)).reshape(
        128, NOB, NP, 2, 128
    ).astype(F8NP)
    invd = np.ascontiguousarray(
        np.broadcast_to(
            (1.0 / np.arange(1, L + 1, dtype=np.float32))[None, :], (128, L)
        )
    ).astype(BFNP)
    biash = np.ascontiguousarray(b_gate.astype(np.float32).reshape(NOB, 128).T)
    return wq, invd, biash


def kernel(inputs: np.ndarray, W_gate: np.ndarray, b_gate: np.ndarray, **run_kwargs):
    inputs = np.asarray(inputs, dtype=np.float32)
    W_gate = np.asarray(W_gate, dtype=np.float32)
    b_gate = np.asarray(b_gate, dtype=np.float32)
    assert inputs.shape == (B, L, D)

    wq, invd, biash = prep_shared(W_gate, b_gate)
    in_maps = []
    for c in range(B):
        xT_c = np.ascontiguousarray(inputs[c].T).reshape(NJ, 128, L)
        in_maps.append({"xT": xT_c, "wq": wq, "invd": invd, "biash": biash})

    nc = build_nc()
    res = bass_utils.run_bass_kernel_spmd(
        nc, in_maps, core_ids=list(range(B)), **run_kwargs
    )

    gating = np.empty((B, L, D), dtype=np.float32)
    average = np.empty((B, L, D), dtype=np.float32)
    for c in range(B):
        gating[c] = res.results[c]["gatT"].astype(np.float32).reshape(D, L).T
        average[c] = res.results[c]["avgT"].astype(np.float32).reshape(D, L).T
    if run_kwargs:
        _CACHE["last_results"] = res
    return gating, average


# revision 62
# speedup vs baseline: 1.4317x; 1.3841x over previous
"""Trainium2 Bass kernel for nn_AverageAttention (B=8, L=2048, D=1024).

Math (per batch b):
    avg[t]  = cumsum(x, axis=t)[t] / (t+1)
    g       = concat([x, avg], -1) @ W_gate.T + b_gate        # (L, 2*D)
    out     = sigmoid(g[:, :D]) * x + sigmoid(g[:, D:]) * avg

Strategy: batch-parallel over 8 NeuronCores (one sequence per core), W_gate
replicated. On-chip layout is transposed (feature-on-partition,
token-on-free) so the cumulative sum is one DVE tensor_tensor_scan per
128-feature chunk.

The gating matmul runs in fp8-e4m3 with MatmulPerfMode.DoubleRow (two
128-row contraction chunks per instruction; measured ~795ns per
K=2048/N=512/M=128 accumulation group on HW vs ~3950ns for bf16). Contraction
chunk m pairs (x_m, avg_m). The whole W (4MB fp8) lives in SBUF, loaded once
per rep. Accuracy: fp8 operand quantization gives ~1.3e-2 rel on the gating
output (threshold 2e-2); avg path stays fp32-scan/bf16-store (~3e-4).

Outputs cross HBM as bf16 (halves store traffic; ~0.1% rounding), upcast to
fp32 on the host. All DMA rides the otherwise-idle sync (SP) HWDGE ring,
ordered: [W pair0 | invd | bias | x0..x7 | W rest (j-major) | avg stores |
gat stores] so x loads are never head-blocked.

Engine placement (HW-measured; queue POSITION matters more than nominal op
cost): every matmul group needs all 16 contraction chunks, so the kernel is
gated by when the last avg chunks' fp8 casts land. DVE runs the scans and,
for the final chunk, produces its fp8 slot directly (ct*invd mul, fp8 out)
so it never waits on Pool's queue; Pool runs only the cumsum*invd muls and
the sigma_i*x combine mul; Act runs both families of fp8 casts (x8 + a8) -
it is idle until the first sigmoid (~55us in), whereas on Pool those casts
sat AHEAD of every avg-chain op in its in-order queue and paced the whole
kernel. The sigma_f*avg mul and final add run on DVE after the scans.
Combine/store emission comes after the whole phase-1 chain so the in-order
queues never head-block the critical path.
"""

from contextlib import ExitStack

import ml_dtypes
import numpy as np

import concourse.bass as bass
import concourse.bass_utils as bass_utils
import concourse.mybir as mybir
import concourse.tile as tile
from concourse import bacc
from concourse._compat import with_exitstack
from concourse.bass import ts

B, L, D = 8, 2048, 1024
NJ = D // 128         # 8 feature chunks of x / avg
NOB = 2 * D // 128    # 16 output-feature blocks of g
NP = NJ               # 8 DoubleRow contraction pairs (x_m, avg_m)
TCW = 512             # matmul moving free-dim (1 PSUM bank)
NTC = L // TCW

FP32 = mybir.dt.float32
BF16 = mybir.dt.bfloat16
FP8 = mybir.dt.float8e4

F8NP = ml_dtypes.float8_e4m3
BFNP = ml_dtypes.bfloat16


@with_exitstack
def _tile_body(
    ctx: ExitStack,
    tc: tile.TileContext,
    reps: int = 1,
    no_mm: bool = False,
    no_act: bool = False,
    no_p1: bool = False,
    add_eng: str = "dve",
    minv_eng: str = "pool",
    scan_mode: str = "dve",
    fast_tail: bool = True,
    scan_bf: bool = False,
    ft_n: int = 1,
    x8_act: int = 0,
    a8_act: bool = False,
    st_bufs: int = 3,
    mul2_eng: str = "dve",
):
    nc = tc.nc

    xT = nc.dram_tensor("xT", (NJ, 128, L), FP32, kind="ExternalInput").ap()
    wq = nc.dram_tensor("wq", (128, NOB, NP, 2, 128), FP8, kind="ExternalInput").ap()
    invd = nc.dram_tensor("invd", (128, L), BF16, kind="ExternalInput").ap()
    biash = nc.dram_tensor("biash", (128, NOB), FP32, kind="ExternalInput").ap()
    avgT = nc.dram_tensor("avgT", (NJ, 128, L), BF16, kind="ExternalOutput").ap()
    gatT = nc.dram_tensor("gatT", (NJ, 128, L), BF16, kind="ExternalOutput").ap()

    const_pool = ctx.enter_context(tc.tile_pool(name="const", bufs=1))
    w_pool = ctx.enter_context(tc.tile_pool(name="w", bufs=1))
    cat_pool = ctx.enter_context(tc.tile_pool(name="cat", bufs=NP))
    abf_pool = ctx.enter_context(tc.tile_pool(name="abf", bufs=NJ))
    x_pool = ctx.enter_context(tc.tile_pool(name="x", bufs=NJ))
    ct_pool = ctx.enter_context(tc.tile_pool(name="ct", bufs=2))
    st_pool = ctx.enter_context(tc.tile_pool(name="st", bufs=st_bufs))
    gt_pool = ctx.enter_context(
        tc.tile_pool(name="gt", bufs=1 if st_bufs >= 4 else 2)
    )
    tmp_pool = ctx.enter_context(tc.tile_pool(name="tmp", bufs=1))
    psum_pool = ctx.enter_context(tc.tile_pool(name="psum", bufs=8, space="PSUM"))

    invd_sb = const_pool.tile([128, L], BF16, tag="invd")
    bias_sb = const_pool.tile([128, NOB], FP32, tag="bias")

    for _rep in range(reps):
        w_sb = w_pool.tile([128, NOB, NP, 2, 128], FP8, name="w_sb", tag="w_sb")
        cats = [
            cat_pool.tile([128, 2, L], FP8, tag="cat", name=f"cat{m}")
            for m in range(NP)
        ]
        abfs = [
            abf_pool.tile([128, L], BF16, tag="abf", name=f"abf{j}") for j in range(NJ)
        ]

        # --- sync-ring head: x0 first (the scan chain is the critical path),
        # then first W pair + constants, then the rest of x ---
        xts = []
        for j in range(NJ):
            xt = x_pool.tile([128, L], FP32, name="xt", tag="xt")
            nc.sync.dma_start(xt[:], xT[j])
            xts.append(xt)
            # x-half fp8 casts, paced only by the x DMAs. Act is idle until
            # the first sigmoid (~55us in), so it can absorb some to relieve
            # Pool's in-order queue.
            if j < x8_act:
                nc.scalar.copy(cats[j][:, 0, :], xt[:])
            else:
                nc.gpsimd.tensor_copy(cats[j][:, 0, :], xt[:])
            if j == 0:
                nc.sync.dma_start(w_sb[:, 0], wq[:, 0])
                nc.sync.dma_start(w_sb[:, NJ], wq[:, NJ])
                if _rep == 0:
                    nc.sync.dma_start(invd_sb[:], invd[:])
                    nc.sync.dma_start(bias_sb[:], biash[:])
        # remaining W, j-major so pair j's tiles land just before needed
        for j in range(1, NJ):
            nc.sync.dma_start(w_sb[:, j], wq[:, j])
            nc.sync.dma_start(w_sb[:, NJ + j], wq[:, NJ + j])

        # --- phase 1: the a8_7 critical chain.
        # DVE runs only scans; Pool only the avg muls; Act casts avg->fp8.
        # Everything else (combine, stores) is emitted after, so the in-order
        # queues never delay the last cat chunk the matmuls wait on.
        for j in range(NJ):
            xt = xts[j]
            if no_p1:
                nc.gpsimd.memset(cats[j][:], 0.25)
                nc.vector.tensor_copy(abfs[j][:], xt[:])
                nc.sync.dma_start(avgT[j], abfs[j][:])
                continue
            ct = ct_pool.tile([128, L], BF16 if scan_bf else FP32, name="ct", tag="ct")
            nc.vector.tensor_tensor_scan(
                ct[:], xt[:], xt[:], 0.0, mybir.AluOpType.add, mybir.AluOpType.bypass
            )
            if fast_tail and j >= NJ - ft_n:
                # last chunk gates all matmul groups: produce its fp8 slot
                # directly on the DVE (one mul, fp8 out) instead of the
                # Pool mul->cast chain; the bf16 copy for store/combine is
                # off the critical path and follows on the DVE.
                nc.vector.tensor_mul(cats[j][:, 1, :], ct[:], invd_sb[:])
                nc.vector.tensor_mul(abfs[j][:], ct[:], invd_sb[:])
            elif minv_eng == "dve":
                nc.vector.tensor_mul(abfs[j][:], ct[:], invd_sb[:])
                nc.gpsimd.tensor_copy(cats[j][:, 1, :], abfs[j][:])
            else:
                nc.gpsimd.tensor_mul(abfs[j][:], ct[:], invd_sb[:])
                if a8_act:
                    nc.scalar.copy(cats[j][:, 1, :], abfs[j][:])
                else:
                    nc.gpsimd.tensor_copy(cats[j][:, 1, :], abfs[j][:])
            nc.sync.dma_start(avgT[j], abfs[j][:])

        if no_mm:
            for j in range(NJ):
                gt = gt_pool.tile([128, L], BF16, name="gt", tag="gt")
                nc.vector.tensor_mul(gt[:], xts[j][:], abfs[j][:])
                nc.sync.dma_start(gatT[j], gt[:])
            continue

        # --- phase 2: DoubleRow fp8 matmul, sigmoid evac, gate combine ---
        for j in range(NJ):
            sts = []
            for ob in (j, NJ + j):
                st = st_pool.tile([128, L], BF16, name="st", tag="st")
                for tcx in range(NTC):
                    s = ts(tcx, TCW)
                    ps = psum_pool.tile([128, TCW], FP32, name="ps", tag="ps")
                    for m in range(NP):
                        nc.tensor.matmul(
                            ps[:],
                            w_sb[:, ob, m],
                            cats[m][:, :, s],
                            start=(m == 0),
                            stop=(m == NP - 1),
                            perf_mode=mybir.MatmulPerfMode.DoubleRow,
                        )
                    if no_act:
                        nc.scalar.copy(st[:, s], ps[:])
                    else:
                        nc.scalar.activation(
                            st[:, s],
                            ps[:],
                            mybir.ActivationFunctionType.Sigmoid,
                            bias=bias_sb[:, ob : ob + 1],
                        )
                sts.append(st)
            st_i, st_f = sts
            gt = gt_pool.tile([128, L], BF16, name="gt", tag="gt")
            tmp = tmp_pool.tile([128, L], BF16, name="tmp", tag="tmp")
            mul2_nc = nc.gpsimd if mul2_eng == "pool" else nc.vector
            if j == NJ - 1:
                # per-chunk tail to shorten the serial path after the last matmul
                for tcx in range(NTC):
                    s = ts(tcx, TCW)
                    nc.gpsimd.tensor_mul(tmp[:, s], st_i[:, s], xts[j][:, s])
                    nc.vector.tensor_mul(gt[:, s], st_f[:, s], abfs[j][:, s])
                    nc.vector.tensor_add(gt[:, s], gt[:, s], tmp[:, s])
                    nc.sync.dma_start(gatT[j][:, s], gt[:, s])
            else:
                nc.gpsimd.tensor_mul(tmp[:], st_i[:], xts[j][:])
                mul2_nc.tensor_mul(gt[:], st_f[:], abfs[j][:])
                if add_eng == "pool":
                    nc.gpsimd.tensor_add(gt[:], gt[:], tmp[:])
                else:
                    nc.vector.tensor_add(gt[:], gt[:], tmp[:])
                nc.sync.dma_start(gatT[j], gt[:])


_CACHE: dict = {}


def build_nc(reps: int | None = None):
    import os as _os

    if reps is None:
        reps = int(_os.environ.get("KREPS", "1"))
    no_mm = _os.environ.get("KNOMM", "0") == "1"
    no_act = _os.environ.get("KNOACT", "0") == "1"
    no_p1 = _os.environ.get("KNOP1", "0") == "1"
    add_eng = _os.environ.get("KADD", "pool")
    minv_eng = _os.environ.get("KMINV", "pool")
    scan_mode = _os.environ.get("KSCAN", "dve")
    fast_tail = _os.environ.get("KFT", "1") == "1"
    scan_bf = _os.environ.get("KSCBF", "0") == "1"
    ft_n = int(_os.environ.get("KFTN", "1"))
    x8_act = int(_os.environ.get("KX8A", "8"))
    a8_act = _os.environ.get("KA8A", "1") == "1"
    st_bufs = int(_os.environ.get("KSTB", "3"))
    mul2_eng = _os.environ.get("KMUL2", "dve")
    key = ("nc", reps, no_mm, no_act, no_p1, add_eng, minv_eng, scan_mode,
           fast_tail, scan_bf, ft_n, x8_act, a8_act, st_bufs, mul2_eng)
    if key not in _CACHE:
        nc = bacc.Bacc(
            "TRN2",
            target_bir_lowering=False,
            debug=False,
            enable_asserts=True,
            num_devices=B,
        )
        with tile.TileContext(nc) as t:
            _tile_body(
                t, reps=reps, no_mm=no_mm, no_act=no_act, no_p1=no_p1,
                add_eng=add_eng, minv_eng=minv_eng, scan_mode=scan_mode,
                fast_tail=fast_tail, scan_bf=scan_bf, ft_n=ft_n,
                x8_act=x8_act, a8_act=a8_act, st_bufs=st_bufs,
                mul2_eng=mul2_eng,
            )
        nc.compile()
        _CACHE[key] = nc
    return _CACHE[key]


# contraction chunk order: pair m slot 0 = x chunk m, slot 1 = avg chunk m
KC_LIST = [c for m in range(NP) for c in (m, NJ + m)]


def prep_shared(W_gate: np.ndarray, b_gate: np.ndarray):
    # wq[p, ob, m, s, o] = W_gate[128*ob + o, 128*kc(m,s) + p]
    arr = np.ascontiguousarray(W_gate.astype(np.float32)).T.reshape(16, 128, NOB, 128)
    # arr[KC_LIST] is (ms, p, ob, o) -> want (p, ob, ms, o)
    wq = np.ascontiguousarray(arr[KC_LIST].transpose(1, 2, 0, 3)).reshape(
        128, NOB, NP, 2, 128
    ).astype(F8NP)
    invd = np.ascontiguousarray(
        np.broadcast_to(
            (1.0 / np.arange(1, L + 1, dtype=np.float32))[None, :], (128, L)
        )
    ).astype(BFNP)
    biash = np.ascontiguousarray(b_gate.astype(np.float32).reshape(NOB, 128).T)
    return wq, invd, biash


def kernel(inputs: np.ndarray, W_gate: np.ndarray, b_gate: np.ndarray, **run_kwargs):
    inputs = np.asarray(inputs, dtype=np.float32)
    W_gate = np.asarray(W_gate, dtype=np.float32)
    b_gate = np.asarray(b_gate, dtype=np.float32)
    assert inputs.shape == (B, L, D)

    wq, invd, biash = prep_shared(W_gate, b_gate)
    in_maps = []
    for c in range(B):
        xT_c = np.ascontiguousarray(inputs[c].T).reshape(NJ, 128, L)
        in_maps.append({"xT": xT_c, "wq": wq, "invd": invd, "biash": biash})

    nc = build_nc()
    res = bass_utils.run_bass_kernel_spmd(
        nc, in_maps, core_ids=list(range(B)), **run_kwargs
    )

    gating = np.empty((B, L, D), dtype=np.float32)
    average = np.empty((B, L, D), dtype=np.float32)
    for c in range(B):
        gating[c] = res.results[c]["gatT"].astype(np.float32).reshape(D, L).T
        average[c] = res.results[c]["avgT"].astype(np.float32).reshape(D, L).T
    if run_kwargs:
        _CACHE["last_results"] = res
    return gating, average
